# revision 19
# baseline (speedup 1.0000x reference)
"""Trainium2 Bass kernel for nn_DiagonalRefine (8-core SPMD).

Math: the reference extracts the main diagonal of feat [2,256,512,512],
runs grouped-conv1d(k=3,g=8)+GELU, dense-conv1d(k=3)+GELU on it, embeds
the result back on the diagonal of a zero image, then depthwise 3x3-blurs
it. The blur of a diagonal-only image is zero outside 5 diagonals:
  out[i, i+d] for d in [-2..2], built from 9 per-channel blur weights and
  sig[i-1], sig[i], sig[i+1].

Sharding: rows are split 8 ways (64 rows/core, full width). The host
pre-gathers the diagonal neighborhood (70 values per (b,c)) so the whole
input arrives in ONE const-table DMA. Both convs run as PE matmuls
(weights pre-laid-out as [ci, k, h, co] slabs, block-diagonal for the
grouped conv), exact GELU on ScalarE, band construction on VectorE.

Output: each row r of a core's 64-row slab is nonzero only at columns
r-2..r+2. The slab is written as two disjoint DRAM tensors so every DMA
descriptor is a large contiguous run and no tiny scatters exist:
  out_band: cols [0,128) of each 516-wide padded row. Built in SBUF: the
    band value (i, d) lands at free offset 129*i + d, which inside the
    128-wide row i is column i+d (self-aligning stride trick). One DMA,
    32KB descriptors.
  out_zero: cols [128,516): pure zeros, DMA'd from a single 388-float
    zero row with a stride-0 source AP (no giant memset). 1552B
    descriptors, starts ~3us into the kernel.
The host unshard stitches band+zero column ranges into global columns.

Wait-slot note: PE Matmult carries a single HW sync-wait slot, so all
constants (incl. diagonals) arrive in ONE DMA and dummy ops observe its
semaphore on PE/ACT/DVE first; PSUM tiles get dedicated banks.
"""

import sys

for _p in ("/opt/trn_rl_repo",):
    if _p not in sys.path:
        sys.path.append(_p)

import numpy as np

import concourse.bass as bass
import concourse.mybir as mybir
from concourse import tile
from concourse.bass_utils import run_bass_kernel_spmd
from bass_rust import add_dep_helper

# ---- problem geometry (hardcoded; see spec) --------------------------------
B = 2
C = 256
L = 512
NCORES = 8
RB = L // NCORES          # 64 rows per core
T = RB + 6                # 70 diag positions (halo 3 each side)
M = T - 2                 # 68 mid positions
S = M - 2                 # 66 sig positions
NQ = 4                    # (batch, channel-half) quarters
BW = 128                  # band-region cols per row (512B descriptors)
ZW = 388                  # zero-region cols per row
WPAD = BW + ZW            # 516 logical padded row width
IMG_B = RB * BW           # 8192 band elems per (partition, quarter)
IMG_Z = RB * ZW           # 24832 zero elems per (partition, quarter)
OUTB_ELEMS = NQ * 128 * IMG_B   # 4,194,304 (16 MiB)
OUTZ_ELEMS = NQ * 128 * IMG_Z   # 12,713,984 (48.5 MiB)
FP32 = mybir.dt.float32

# packed const-table per-partition layout (f32 offsets)
W1_OFF = 0                # [6C]   (k,h) -> slab of C cout
W2_OFF = 6 * C            # [6C]
WB_OFF = 12 * C           # [18]   (h, ki*3+kj)
B1_OFF = WB_OFF + 18      # [2]
B2_OFF = B1_OFF + 2       # [2]
MSK_OFF = B2_OFF + 2      # [2M]   h-mask [M], s-mask [S] (padded to M)
DIAG_OFF = MSK_OFF + 2 * M  # [4T]  per-quarter diagonal (host pre-gathered)
CT_FREE = DIAG_OFF + NQ * T  # 3510

_cache = {}


def _build_nc(act=mybir.ActivationFunctionType.Gelu):
    nc = bass.Bass()
    wtab = nc.declare_dram_parameter("wtab", [128 * CT_FREE], FP32, isOutput=False)
    outb = nc.declare_dram_parameter("out_band", [OUTB_ELEMS], FP32, isOutput=True)
    outz = nc.declare_dram_parameter("out_zero", [OUTZ_ELEMS], FP32, isOutput=True)

    mul = mybir.AluOpType.mult
    add = mybir.AluOpType.add

    with tile.TileContext(nc) as tc:
        with (
            tc.tile_pool(name="const", bufs=1) as cpool,
            tc.tile_pool(name="zero", bufs=1) as zpool,
            tc.tile_pool(name="work", bufs=4) as wpool,
            tc.tile_pool(name="band", bufs=1) as bpool,
            tc.tile_pool(name="mpsum", bufs=4, space=bass.MemorySpace.PSUM) as mpool,
            tc.tile_pool(name="spsum", bufs=4, space=bass.MemorySpace.PSUM) as spool,
        ):
            # ---- zero stream first: memset a 32-row zero block, then
            # 48.5 MiB of zeros flow from a stride-0 source starting ~11us
            # into the kernel. RT rows per descriptor => 49.6KB descriptors
            # (1-row descriptors measured only 248 GB/s from per-packet
            # overhead; big descriptors run at full HBM write rate).
            RT = 16
            HALF = RT * ZW
            TR = 4            # tiny-tile rows
            CH = 8            # rows streamed from the tiny tile at start
            ztiny = zpool.tile([128, TR * ZW], FP32, tag="ztiny")
            ztile = zpool.tile([128, 2 * HALF], FP32, tag="ztile")
            tmemset = nc.gpsimd.memset(ztiny[:], 0.0)
            gmemset = nc.gpsimd.memset(ztile[:, HALF:2 * HALF], 0.0)
            zmemset = nc.vector.memset(ztile[:, 0:HALF], 0.0)
            zdmas = []
            # stage 1 (rows 0..CH of quarter 0): 4-row descriptors from the
            # tiny tile (memset in ~1.3us) -- flowing ~5us before the big
            # memset halves (DVE + GpSimd in parallel) finish.
            zdmas.append(nc.sync.dma_start(
                bass.AP(outz, 0, [[IMG_Z, 128], [TR * ZW, CH // TR], [1, TR * ZW]]),
                bass.AP(ztiny.tensor, 0, [[TR * ZW, 128], [0, CH // TR], [1, TR * ZW]]),
            ))
            # two 16-row absorber DMAs: each trigger's single wait slot
            # observes one big-memset semaphore, so every later trigger can
            # source the full tile with no waits. NOTE: every DMA keeps >=2
            # descriptors per partition (3-dim AP) -- single-descriptor-per-
            # partition DMAs measured only ~270 GB/s vs ~420 for 2+.
            for hh in range(2):
                zdmas.append(nc.sync.dma_start(
                    bass.AP(outz, (CH + hh * RT) * ZW,
                            [[IMG_Z, 128], [(RT // 2) * ZW, 2], [1, (RT // 2) * ZW]]),
                    bass.AP(ztile.tensor, hh * HALF,
                            [[2 * HALF, 128], [(RT // 2) * ZW, 2], [1, (RT // 2) * ZW]]),
                ))
            # quarter-0 remainder (rows CH+32..64): 12-row descriptors x2
            QR = (RB - CH - 2 * RT) // 2
            zdmas.append(nc.sync.dma_start(
                bass.AP(outz, (CH + 2 * RT) * ZW,
                        [[IMG_Z, 128], [QR * ZW, 2], [1, QR * ZW]]),
                bass.AP(ztile.tensor, 0,
                        [[2 * HALF, 128], [0, 2], [1, QR * ZW]]),
            ))
            # quarters 1-3: one DMA each, 32-row descriptors (full tile).
            # 7 zero DMAs + 1 band DMA = all 8 DMAHW sem lanes, no wrap (a
            # 9th HWDGE DMA would reuse lane 0 and stall on its completion).
            for q in range(1, NQ):
                zdmas.append(nc.sync.dma_start(
                    bass.AP(outz, q * 128 * IMG_Z,
                            [[IMG_Z, 128], [2 * HALF, RB // (2 * RT)], [1, 2 * HALF]]),
                    bass.AP(ztile.tensor, 0,
                            [[2 * HALF, 128], [0, RB // (2 * RT)], [1, 2 * HALF]]),
                ))

            # band slab: 4 quarters x [64 rows x 128 cols]; memset while the
            # zero stream drains (DVE is otherwise idle).
            slab = zpool.tile([128, NQ * IMG_B], FP32, tag="slab")
            nc.vector.memset(slab[:], 0.0)

            # ---- all constants + diagonals in ONE DMA (single sem source) --
            ctile = cpool.tile([128, CT_FREE], FP32, tag="ctile")
            cdma = nc.gpsimd.dma_start(
                ctile[:], bass.AP(wtab, 0, [[CT_FREE, 128], [1, CT_FREE]])
            )

            # observer ops: let PE/ACT see the const DMA's semaphore before
            # any real consumer, keeping later ops at <=1 sync wait.
            mps = [mpool.tile([128, M], FP32, tag="mps", name=f"mps{i}") for i in range(4)]
            sps = [spool.tile([128, S], FP32, tag="sps", name=f"sps{i}") for i in range(4)]
            scratch = cpool.tile([1, 1], FP32, tag="scratch")
            scratch2 = cpool.tile([1, 1], FP32, tag="scratch2")
            with tc.high_priority():
                nc.tensor.matmul(mps[0][0:2, 0:2], ctile[:, 0:2], ctile[:, 0:2],
                                 start=True, stop=True, skip_group_check=True)
                nc.scalar.copy(scratch[:], ctile[0:1, 0:1])
            # DVE observer AFTER the memsets (so zero DMAs are not gated on
            # the const DMA) but before compute DVE ops; emission order is
            # schedule order per engine, and same-engine dep edges would
            # become a second (illegal) sync wait.
            nc.vector.tensor_copy(scratch2[:], ctile[0:1, 0:1])

            def wslab(off, k, h, co_h):
                # lhsT chunk [128 ci, 128 co]
                s = off + (k * 2 + h) * C + co_h * 128
                return ctile[:, s:s + 128]

            mh_bc = ctile[:, MSK_OFF:MSK_OFF + M]
            ms_bc = ctile[:, MSK_OFF + M:MSK_OFF + M + S]

            bandall = bpool.tile([128, NQ * RB * 5], FP32, tag="bandall")
            for b in range(B):
                hsb = []
                for h in range(2):
                    q = b * 2 + h
                    diag = ctile[:, DIAG_OFF + q * T:DIAG_OFF + (q + 1) * T]
                    mp = mps[q]
                    for k in range(3):
                        nc.tensor.matmul(
                            mp[:], wslab(W1_OFF, k, h, h), diag[:, k:k + M],
                            start=(k == 0), stop=(k == 2),
                            skip_group_check=(b == 0 and h == 0),
                        )
                    hcur = wpool.tile([128, M], FP32, tag="h")
                    nc.scalar.activation(
                        hcur[:], mp[:], act,
                        bias=ctile[:, B1_OFF + h:B1_OFF + h + 1],
                    )
                    nc.vector.tensor_mul(hcur[:], hcur[:], mh_bc)
                    hsb.append(hcur)

                for h in range(2):
                    sp = sps[2 * b + h]
                    first = True
                    for k in range(3):
                        for ci_h in range(2):
                            last_mm = nc.tensor.matmul(
                                sp[:], wslab(W2_OFF, k, ci_h, h),
                                hsb[ci_h][:, k:k + S],
                                start=first, stop=(k == 2 and ci_h == 1),
                            )
                            first = False
                    sig = wpool.tile([128, S], FP32, tag="sig")
                    last_gelu = nc.scalar.activation(
                        sig[:], sp[:], act,
                        bias=ctile[:, B2_OFF + h:B2_OFF + h + 1],
                    )
                    nc.vector.tensor_mul(sig[:], sig[:], ms_bc)

                    # band construction: 5 interleaved columns per quarter
                    q = b * 2 + h
                    bv = bandall[:, q * RB * 5:(q + 1) * RB * 5].rearrange(
                        "p (i d) -> p i d", d=5)
                    s0 = sig[:, 0:RB].unsqueeze(2)      # sig[i-1]
                    s1 = sig[:, 1:RB + 1].unsqueeze(2)  # sig[i]
                    s2 = sig[:, 2:RB + 2].unsqueeze(2)  # sig[i+1]

                    def wb(ki, kj):
                        s = WB_OFF + h * 9 + ki * 3 + kj
                        return ctile[:, s:s + 1]

                    tmp = bpool.tile([128, RB], FP32, tag="tmp")
                    tmpv = tmp[:].unsqueeze(2)
                    tmp2 = bpool.tile([128, RB], FP32, tag="tmp2")
                    tmp2v = tmp2[:].unsqueeze(2)

                    # d=-2: w[0,2]*s0 ; d=+2: w[2,0]*s2
                    nc.vector.tensor_scalar_mul(bv[:, :, 0:1], s0, wb(0, 2))
                    nc.vector.tensor_scalar_mul(bv[:, :, 4:5], s2, wb(2, 0))
                    # d=-1: w[0,1]*s0 + w[1,2]*s1
                    nc.vector.tensor_scalar_mul(tmpv, s1, wb(1, 2))
                    nc.vector.scalar_tensor_tensor(bv[:, :, 1:2], s0, wb(0, 1), tmpv, mul, add)
                    # d=+1: w[1,0]*s1 + w[2,1]*s2
                    nc.vector.tensor_scalar_mul(tmpv, s2, wb(2, 1))
                    nc.vector.scalar_tensor_tensor(bv[:, :, 3:4], s1, wb(1, 0), tmpv, mul, add)
                    # d=0: w[0,0]*s0 + w[1,1]*s1 + w[2,2]*s2
                    nc.vector.tensor_scalar_mul(tmp2v, s0, wb(0, 0))
                    nc.vector.scalar_tensor_tensor(tmpv, s1, wb(1, 1), tmp2v, mul, add)
                    nc.vector.scalar_tensor_tensor(bv[:, :, 2:3], s2, wb(2, 2), tmpv, mul, add)

            # place band values into the slab: (i, d) -> 129*i + d, which is
            # column i+d of the 128-wide row i (self-aligning stride trick).
            last_copy = None
            for q in range(NQ):
                last_copy = nc.vector.tensor_copy(
                    bass.AP(slab.tensor, q * IMG_B,
                            [[NQ * IMG_B, 128], [BW + 1, RB], [1, 5]]),
                    bandall[:, q * RB * 5:(q + 1) * RB * 5].rearrange(
                        "p (i d) -> p i d", d=5),
                )

            # single band DMA: 4 x 32KB contiguous descriptors per partition
            bdma = nc.sync.dma_start(
                bass.AP(outb, 0, [[IMG_B, 128], [128 * IMG_B, NQ], [1, IMG_B]]),
                bass.AP(slab.tensor, 0, [[NQ * IMG_B, 128], [IMG_B, NQ], [1, IMG_B]]),
            )

            # ---- tail nop ladders: bring each sequencer's observed clock
            # current one semaphore at a time (every instruction gets at most
            # ONE sync wait), so Tile's final drains need no multi-waits.
            def ladder(eng, deps):
                for dinst in deps:
                    n = eng.nop()
                    add_dep_helper(n.ins, dinst.ins, reason="tail clock catch-up")
            ladder(nc.sync, [cdma] + zdmas + [bdma, last_copy, last_gelu, last_mm])
            ladder(nc.scalar, zdmas + [bdma, last_copy])
            ladder(nc.gpsimd, [cdma] + zdmas + [bdma, last_copy, last_gelu, last_mm])
            ladder(nc.vector, [last_mm, last_gelu] + zdmas + [bdma])
            ladder(nc.tensor, zdmas + [bdma, last_copy, last_gelu])
    return nc


def _prep_shared(w1, b1, w2, b2, w_blur):
    """Pack all weights/consts into the per-partition const table
    [128, CT_FREE]; layout along free dim documented at top of file."""
    ct = np.zeros((128, CT_FREE), np.float32)
    # w1 block-diag [ci_l, (k,h), co]
    w1kh = np.zeros((3, 2, 128, C), np.float32)  # [k, h, ci_l, co]
    gc = C // 8
    for co in range(C):
        g = co // gc
        h, cil0 = divmod(g * gc, 128)
        w1kh[:, h, cil0:cil0 + gc, co] = w1[co].T  # w1[co] is [32,3]
    ct[:, W1_OFF:W1_OFF + 6 * C] = w1kh.transpose(2, 0, 1, 3).reshape(128, 6 * C)
    # w2 dense: [ci_l, k, h, co] = w2[co, h*128+ci_l, k]
    w2r = w2.transpose(1, 2, 0).reshape(2, 128, 3, C).transpose(1, 2, 0, 3)
    ct[:, W2_OFF:W2_OFF + 6 * C] = w2r.reshape(128, 6 * C)
    ct[:, WB_OFF:WB_OFF + 18] = \
        w_blur.reshape(2, 128, 9).transpose(1, 0, 2).reshape(128, 18)
    ct[:, B1_OFF:B1_OFF + 2] = b1.reshape(2, 128).T
    ct[:, B2_OFF:B2_OFF + 2] = b2.reshape(2, 128).T
    return ct


def _prep_core(diagp, ct, g):
    """Fill the per-core const table: edge masks + the 70-wide diagonal
    neighborhood for each (batch, channel-half) quarter."""
    base = g * RB
    mh = np.ones(M, np.float32)
    ms = np.ones(M, np.float32)
    if g == 0:
        mh[0:2] = 0.0
        ms[0] = 0.0
    if g == NCORES - 1:
        mh[M - 2:M] = 0.0
        ms[S - 1] = 0.0
    ctg = ct.copy()
    ctg[:, MSK_OFF:MSK_OFF + M] = mh
    ctg[:, MSK_OFF + M:MSK_OFF + 2 * M] = ms
    for q in range(NQ):
        b, h = divmod(q, 2)
        ctg[:, DIAG_OFF + q * T:DIAG_OFF + (q + 1) * T] = \
            diagp[b, h * 128:(h + 1) * 128, base:base + T]
    return ctg.ravel()


def _run(inputs, trace=False, **kw):
    feat = np.asarray(inputs["feat"], np.float32)
    ct = _prep_shared(
        np.asarray(inputs["w1"], np.float32), np.asarray(inputs["b1"], np.float32),
        np.asarray(inputs["w2"], np.float32), np.asarray(inputs["b2"], np.float32),
        np.asarray(inputs["w_blur"], np.float32),
    )
    # host-side diagonal gather (tiny: [B,C,L] = 1 MiB), zero-padded halo
    diag = np.ascontiguousarray(np.diagonal(feat, axis1=2, axis2=3))  # [B,C,L]
    diagp = np.zeros((B, C, L + 6), np.float32)
    diagp[:, :, 3:L + 3] = diag
    in_maps = [{"wtab": _prep_core(diagp, ct, g)} for g in range(NCORES)]
    if "nc" not in _cache:
        _cache["nc"] = _build_nc()
    res = run_bass_kernel_spmd(
        _cache["nc"], in_maps, core_ids=list(range(NCORES)), trace=trace, **kw
    )
    _cache["last_result"] = res

    full = np.empty((B, C, L, L), np.float32)
    for g in range(NCORES):
        bnd = res.results[g]["out_band"].reshape(B, C, RB, BW)
        zer = res.results[g]["out_zero"].reshape(B, C, RB, ZW)
        rows = slice(g * RB, (g + 1) * RB)
        base = g * RB
        # band col j0 -> global col base-2+j0 ; zero col jz -> base-2+BW+jz
        b_lo = max(0, base - 2)
        j0_lo = b_lo - (base - 2)
        b_hi = min(L, base - 2 + BW)
        full[:, :, rows, b_lo:b_hi] = bnd[:, :, :, j0_lo:j0_lo + (b_hi - b_lo)]
        wz = L - b_hi                      # trailing zeros (<= 386 <= ZW)
        if wz > 0:
            full[:, :, rows, b_hi:L] = zer[:, :, :, 0:wz]
        wl = b_lo                          # leading zeros (<= 446)
        if wl > 0:
            a = min(wl, ZW)
            full[:, :, rows, 0:a] = zer[:, :, :, 0:a]
            if wl > a:
                full[:, :, rows, a:wl] = zer[:, :, :, 0:wl - a]
    return full


def kernel(**inputs):
    return _run(inputs, trace=False)


# revision 20
# speedup vs baseline: 1.0169x; 1.0169x over previous
"""Trainium2 Bass kernel for nn_DiagonalRefine (8-core SPMD).

Math: the reference extracts the main diagonal of feat [2,256,512,512],
runs grouped-conv1d(k=3,g=8)+GELU, dense-conv1d(k=3)+GELU on it, embeds
the result back on the diagonal of a zero image, then depthwise 3x3-blurs
it. The blur of a diagonal-only image is zero outside 5 diagonals:
  out[i, i+d] for d in [-2..2], built from 9 per-channel blur weights and
  sig[i-1], sig[i], sig[i+1].

Sharding: rows are split 8 ways (64 rows/core, full width). The host
pre-gathers the diagonal neighborhood (70 values per (b,c)) so the whole
input arrives in ONE const-table DMA. Both convs run as PE matmuls
(weights pre-laid-out as [ci, k, h, co] slabs, block-diagonal for the
grouped conv), exact GELU on ScalarE, band construction on VectorE.

Output: each row r of a core's 64-row slab is nonzero only at columns
r-2..r+2. The slab is written as two disjoint DRAM tensors so every DMA
descriptor is a large contiguous run and no tiny scatters exist:
  out_band: cols [0,128) of each 516-wide padded row. Built in SBUF: the
    band value (i, d) lands at free offset 129*i + d, which inside the
    128-wide row i is column i+d (self-aligning stride trick). One DMA,
    32KB descriptors.
  out_zero: cols [128,516): pure zeros, DMA'd from a single 388-float
    zero row with a stride-0 source AP (no giant memset). 1552B
    descriptors, starts ~3us into the kernel.
The host unshard stitches band+zero column ranges into global columns.

Wait-slot note: PE Matmult carries a single HW sync-wait slot, so all
constants (incl. diagonals) arrive in ONE DMA and dummy ops observe its
semaphore on PE/ACT/DVE first; PSUM tiles get dedicated banks.
"""

import sys

for _p in ("/opt/trn_rl_repo",):
    if _p not in sys.path:
        sys.path.append(_p)

import numpy as np

import concourse.bass as bass
import concourse.mybir as mybir
from concourse import tile
from concourse.bass_utils import run_bass_kernel_spmd
from bass_rust import add_dep_helper

# ---- problem geometry (hardcoded; see spec) --------------------------------
B = 2
C = 256
L = 512
NCORES = 8
RB = L // NCORES          # 64 rows per core
T = RB + 6                # 70 diag positions (halo 3 each side)
M = T - 2                 # 68 mid positions
S = M - 2                 # 66 sig positions
NQ = 4                    # (batch, channel-half) quarters
BW = 128                  # band-region cols per row (512B descriptors)
ZW = 388                  # zero-region cols per row
WPAD = BW + ZW            # 516 logical padded row width
IMG_B = RB * BW           # 8192 band elems per (partition, quarter)
IMG_Z = RB * ZW           # 24832 zero elems per (partition, quarter)
OUTB_ELEMS = NQ * 128 * IMG_B   # 4,194,304 (16 MiB)
OUTZ_ELEMS = NQ * 128 * IMG_Z   # 12,713,984 (48.5 MiB)
FP32 = mybir.dt.float32

# packed const-table per-partition layout (f32 offsets)
W1_OFF = 0                # [6C]   (k,h) -> slab of C cout
W2_OFF = 6 * C            # [6C]
WB_OFF = 12 * C           # [18]   (h, ki*3+kj)
B1_OFF = WB_OFF + 18      # [2]
B2_OFF = B1_OFF + 2       # [2]
MSK_OFF = B2_OFF + 2      # [2M]   h-mask [M], s-mask [S] (padded to M)
DIAG_OFF = MSK_OFF + 2 * M  # [4T]  per-quarter diagonal (host pre-gathered)
CT_FREE = DIAG_OFF + NQ * T  # 3510

_cache = {}


def _build_nc(act=mybir.ActivationFunctionType.Gelu):
    nc = bass.Bass()
    wtab = nc.declare_dram_parameter("wtab", [128 * CT_FREE], FP32, isOutput=False)
    outb = nc.declare_dram_parameter("out_band", [OUTB_ELEMS], FP32, isOutput=True)
    outz = nc.declare_dram_parameter("out_zero", [OUTZ_ELEMS], FP32, isOutput=True)

    mul = mybir.AluOpType.mult
    add = mybir.AluOpType.add

    with tile.TileContext(nc) as tc:
        with (
            tc.tile_pool(name="const", bufs=1) as cpool,
            tc.tile_pool(name="zero", bufs=1) as zpool,
            tc.tile_pool(name="work", bufs=4) as wpool,
            tc.tile_pool(name="band", bufs=1) as bpool,
            tc.tile_pool(name="mpsum", bufs=4, space=bass.MemorySpace.PSUM) as mpool,
            tc.tile_pool(name="spsum", bufs=4, space=bass.MemorySpace.PSUM) as spool,
        ):
            # ---- zero stream first: memset a 32-row zero block, then
            # 48.5 MiB of zeros flow from a stride-0 source starting ~11us
            # into the kernel. RT rows per descriptor => 49.6KB descriptors
            # (1-row descriptors measured only 248 GB/s from per-packet
            # overhead; big descriptors run at full HBM write rate).
            RT = 16
            HALF = RT * ZW
            CH = 6            # rows streamed from the 1-row tile at start
            ztiny = zpool.tile([128, ZW], FP32, tag="ztiny")
            ztile = zpool.tile([128, 2 * HALF], FP32, tag="ztile")
            tmemset = nc.gpsimd.memset(ztiny[:], 0.0)
            gmemset = nc.gpsimd.memset(ztile[:, HALF:2 * HALF], 0.0)
            zmemset = nc.vector.memset(ztile[:, 0:HALF], 0.0)
            zdmas = []
            # stage 1 (rows 0..CH of quarter 0): 1-row descriptors from the
            # tiny tile -- slower per-packet, but flowing ~5us before the
            # big memset halves (DVE + GpSimd in parallel) finish.
            zdmas.append(nc.sync.dma_start(
                bass.AP(outz, 0, [[IMG_Z, 128], [ZW, CH], [1, ZW]]),
                bass.AP(ztiny.tensor, 0, [[ZW, 128], [0, CH], [1, ZW]]),
            ))
            # two 1-row absorber DMAs: each trigger's single wait slot
            # observes one big-memset semaphore, so every later trigger
            # can source the full tile with no waits.
            for hh in range(2):
                zdmas.append(nc.sync.dma_start(
                    bass.AP(outz, (CH + hh) * ZW, [[IMG_Z, 128], [1, ZW]]),
                    bass.AP(ztile.tensor, hh * HALF, [[2 * HALF, 128], [1, ZW]]),
                ))
            # quarter-0 remainder: 28-row descriptors spanning both halves
            zdmas.append(nc.sync.dma_start(
                bass.AP(outz, (CH + 2) * ZW,
                        [[IMG_Z, 128], [28 * ZW, 2], [1, 28 * ZW]]),
                bass.AP(ztile.tensor, 0, [[2 * HALF, 128], [0, 2], [1, 28 * ZW]]),
            ))
            # quarters 1-3: one DMA each, 32-row descriptors (full tile).
            # 7 zero DMAs + 1 band DMA = all 8 DMAHW sem lanes, no wrap (a
            # 9th HWDGE DMA would reuse lane 0 and stall on its completion).
            for q in range(1, NQ):
                zdmas.append(nc.sync.dma_start(
                    bass.AP(outz, q * 128 * IMG_Z,
                            [[IMG_Z, 128], [2 * HALF, RB // (2 * RT)], [1, 2 * HALF]]),
                    bass.AP(ztile.tensor, 0,
                            [[2 * HALF, 128], [0, RB // (2 * RT)], [1, 2 * HALF]]),
                ))

            # band slab: 4 quarters x [64 rows x 128 cols]; memset while the
            # zero stream drains (DVE is otherwise idle).
            slab = zpool.tile([128, NQ * IMG_B], FP32, tag="slab")
            nc.vector.memset(slab[:], 0.0)

            # ---- all constants + diagonals in ONE DMA (single sem source) --
            ctile = cpool.tile([128, CT_FREE], FP32, tag="ctile")
            cdma = nc.gpsimd.dma_start(
                ctile[:], bass.AP(wtab, 0, [[CT_FREE, 128], [1, CT_FREE]])
            )

            # observer ops: let PE/ACT see the const DMA's semaphore before
            # any real consumer, keeping later ops at <=1 sync wait.
            mps = [mpool.tile([128, M], FP32, tag="mps", name=f"mps{i}") for i in range(4)]
            sps = [spool.tile([128, S], FP32, tag="sps", name=f"sps{i}") for i in range(4)]
            scratch = cpool.tile([1, 1], FP32, tag="scratch")
            scratch2 = cpool.tile([1, 1], FP32, tag="scratch2")
            with tc.high_priority():
                nc.tensor.matmul(mps[0][0:2, 0:2], ctile[:, 0:2], ctile[:, 0:2],
                                 start=True, stop=True, skip_group_check=True)
                nc.scalar.copy(scratch[:], ctile[0:1, 0:1])
            # DVE observer AFTER the memsets (so zero DMAs are not gated on
            # the const DMA) but before compute DVE ops; emission order is
            # schedule order per engine, and same-engine dep edges would
            # become a second (illegal) sync wait.
            nc.vector.tensor_copy(scratch2[:], ctile[0:1, 0:1])

            def wslab(off, k, h, co_h):
                # lhsT chunk [128 ci, 128 co]
                s = off + (k * 2 + h) * C + co_h * 128
                return ctile[:, s:s + 128]

            mh_bc = ctile[:, MSK_OFF:MSK_OFF + M]
            ms_bc = ctile[:, MSK_OFF + M:MSK_OFF + M + S]

            bandall = bpool.tile([128, NQ * RB * 5], FP32, tag="bandall")
            for b in range(B):
                hsb = []
                for h in range(2):
                    q = b * 2 + h
                    diag = ctile[:, DIAG_OFF + q * T:DIAG_OFF + (q + 1) * T]
                    mp = mps[q]
                    for k in range(3):
                        nc.tensor.matmul(
                            mp[:], wslab(W1_OFF, k, h, h), diag[:, k:k + M],
                            start=(k == 0), stop=(k == 2),
                            skip_group_check=(b == 0 and h == 0),
                        )
                    hcur = wpool.tile([128, M], FP32, tag="h")
                    nc.scalar.activation(
                        hcur[:], mp[:], act,
                        bias=ctile[:, B1_OFF + h:B1_OFF + h + 1],
                    )
                    nc.vector.tensor_mul(hcur[:], hcur[:], mh_bc)
                    hsb.append(hcur)

                for h in range(2):
                    sp = sps[2 * b + h]
                    first = True
                    for k in range(3):
                        for ci_h in range(2):
                            last_mm = nc.tensor.matmul(
                                sp[:], wslab(W2_OFF, k, ci_h, h),
                                hsb[ci_h][:, k:k + S],
                                start=first, stop=(k == 2 and ci_h == 1),
                            )
                            first = False
                    sig = wpool.tile([128, S], FP32, tag="sig")
                    last_gelu = nc.scalar.activation(
                        sig[:], sp[:], act,
                        bias=ctile[:, B2_OFF + h:B2_OFF + h + 1],
                    )
                    nc.vector.tensor_mul(sig[:], sig[:], ms_bc)

                    # band construction: 5 interleaved columns per quarter
                    q = b * 2 + h
                    bv = bandall[:, q * RB * 5:(q + 1) * RB * 5].rearrange(
                        "p (i d) -> p i d", d=5)
                    s0 = sig[:, 0:RB].unsqueeze(2)      # sig[i-1]
                    s1 = sig[:, 1:RB + 1].unsqueeze(2)  # sig[i]
                    s2 = sig[:, 2:RB + 2].unsqueeze(2)  # sig[i+1]

                    def wb(ki, kj):
                        s = WB_OFF + h * 9 + ki * 3 + kj
                        return ctile[:, s:s + 1]

                    tmp = bpool.tile([128, RB], FP32, tag="tmp")
                    tmpv = tmp[:].unsqueeze(2)
                    tmp2 = bpool.tile([128, RB], FP32, tag="tmp2")
                    tmp2v = tmp2[:].unsqueeze(2)

                    # d=-2: w[0,2]*s0 ; d=+2: w[2,0]*s2
                    nc.vector.tensor_scalar_mul(bv[:, :, 0:1], s0, wb(0, 2))
                    nc.vector.tensor_scalar_mul(bv[:, :, 4:5], s2, wb(2, 0))
                    # d=-1: w[0,1]*s0 + w[1,2]*s1
                    nc.vector.tensor_scalar_mul(tmpv, s1, wb(1, 2))
                    nc.vector.scalar_tensor_tensor(bv[:, :, 1:2], s0, wb(0, 1), tmpv, mul, add)
                    # d=+1: w[1,0]*s1 + w[2,1]*s2
                    nc.vector.tensor_scalar_mul(tmpv, s2, wb(2, 1))
                    nc.vector.scalar_tensor_tensor(bv[:, :, 3:4], s1, wb(1, 0), tmpv, mul, add)
                    # d=0: w[0,0]*s0 + w[1,1]*s1 + w[2,2]*s2
                    nc.vector.tensor_scalar_mul(tmp2v, s0, wb(0, 0))
                    nc.vector.scalar_tensor_tensor(tmpv, s1, wb(1, 1), tmp2v, mul, add)
                    nc.vector.scalar_tensor_tensor(bv[:, :, 2:3], s2, wb(2, 2), tmpv, mul, add)

            # place band values into the slab: (i, d) -> 129*i + d, which is
            # column i+d of the 128-wide row i (self-aligning stride trick).
            last_copy = None
            for q in range(NQ):
                last_copy = nc.vector.tensor_copy(
                    bass.AP(slab.tensor, q * IMG_B,
                            [[NQ * IMG_B, 128], [BW + 1, RB], [1, 5]]),
                    bandall[:, q * RB * 5:(q + 1) * RB * 5].rearrange(
                        "p (i d) -> p i d", d=5),
                )

            # single band DMA: 4 x 32KB contiguous descriptors per partition
            bdma = nc.sync.dma_start(
                bass.AP(outb, 0, [[IMG_B, 128], [128 * IMG_B, NQ], [1, IMG_B]]),
                bass.AP(slab.tensor, 0, [[NQ * IMG_B, 128], [IMG_B, NQ], [1, IMG_B]]),
            )

            # ---- tail nop ladders: bring each sequencer's observed clock
            # current one semaphore at a time (every instruction gets at most
            # ONE sync wait), so Tile's final drains need no multi-waits.
            def ladder(eng, deps):
                for dinst in deps:
                    n = eng.nop()
                    add_dep_helper(n.ins, dinst.ins, reason="tail clock catch-up")
            ladder(nc.sync, [cdma] + zdmas + [bdma, last_copy, last_gelu, last_mm])
            ladder(nc.scalar, zdmas + [bdma, last_copy])
            ladder(nc.gpsimd, [cdma] + zdmas + [bdma, last_copy, last_gelu, last_mm])
            ladder(nc.vector, [last_mm, last_gelu] + zdmas + [bdma])
            ladder(nc.tensor, zdmas + [bdma, last_copy, last_gelu])
    return nc


def _prep_shared(w1, b1, w2, b2, w_blur):
    """Pack all weights/consts into the per-partition const table
    [128, CT_FREE]; layout along free dim documented at top of file."""
    ct = np.zeros((128, CT_FREE), np.float32)
    # w1 block-diag [ci_l, (k,h), co]
    w1kh = np.zeros((3, 2, 128, C), np.float32)  # [k, h, ci_l, co]
    gc = C // 8
    for co in range(C):
        g = co // gc
        h, cil0 = divmod(g * gc, 128)
        w1kh[:, h, cil0:cil0 + gc, co] = w1[co].T  # w1[co] is [32,3]
    ct[:, W1_OFF:W1_OFF + 6 * C] = w1kh.transpose(2, 0, 1, 3).reshape(128, 6 * C)
    # w2 dense: [ci_l, k, h, co] = w2[co, h*128+ci_l, k]
    w2r = w2.transpose(1, 2, 0).reshape(2, 128, 3, C).transpose(1, 2, 0, 3)
    ct[:, W2_OFF:W2_OFF + 6 * C] = w2r.reshape(128, 6 * C)
    ct[:, WB_OFF:WB_OFF + 18] = \
        w_blur.reshape(2, 128, 9).transpose(1, 0, 2).reshape(128, 18)
    ct[:, B1_OFF:B1_OFF + 2] = b1.reshape(2, 128).T
    ct[:, B2_OFF:B2_OFF + 2] = b2.reshape(2, 128).T
    return ct


def _prep_core(diagp, ct, g):
    """Fill the per-core const table: edge masks + the 70-wide diagonal
    neighborhood for each (batch, channel-half) quarter."""
    base = g * RB
    mh = np.ones(M, np.float32)
    ms = np.ones(M, np.float32)
    if g == 0:
        mh[0:2] = 0.0
        ms[0] = 0.0
    if g == NCORES - 1:
        mh[M - 2:M] = 0.0
        ms[S - 1] = 0.0
    ctg = ct.copy()
    ctg[:, MSK_OFF:MSK_OFF + M] = mh
    ctg[:, MSK_OFF + M:MSK_OFF + 2 * M] = ms
    for q in range(NQ):
        b, h = divmod(q, 2)
        ctg[:, DIAG_OFF + q * T:DIAG_OFF + (q + 1) * T] = \
            diagp[b, h * 128:(h + 1) * 128, base:base + T]
    return ctg.ravel()


def _run(inputs, trace=False, **kw):
    feat = np.asarray(inputs["feat"], np.float32)
    ct = _prep_shared(
        np.asarray(inputs["w1"], np.float32), np.asarray(inputs["b1"], np.float32),
        np.asarray(inputs["w2"], np.float32), np.asarray(inputs["b2"], np.float32),
        np.asarray(inputs["w_blur"], np.float32),
    )
    # host-side diagonal gather (tiny: [B,C,L] = 1 MiB), zero-padded halo
    diag = np.ascontiguousarray(np.diagonal(feat, axis1=2, axis2=3))  # [B,C,L]
    diagp = np.zeros((B, C, L + 6), np.float32)
    diagp[:, :, 3:L + 3] = diag
    in_maps = [{"wtab": _prep_core(diagp, ct, g)} for g in range(NCORES)]
    if "nc" not in _cache:
        _cache["nc"] = _build_nc()
    res = run_bass_kernel_spmd(
        _cache["nc"], in_maps, core_ids=list(range(NCORES)), trace=trace, **kw
    )
    _cache["last_result"] = res

    full = np.empty((B, C, L, L), np.float32)
    for g in range(NCORES):
        bnd = res.results[g]["out_band"].reshape(B, C, RB, BW)
        zer = res.results[g]["out_zero"].reshape(B, C, RB, ZW)
        rows = slice(g * RB, (g + 1) * RB)
        base = g * RB
        # band col j0 -> global col base-2+j0 ; zero col jz -> base-2+BW+jz
        b_lo = max(0, base - 2)
        j0_lo = b_lo - (base - 2)
        b_hi = min(L, base - 2 + BW)
        full[:, :, rows, b_lo:b_hi] = bnd[:, :, :, j0_lo:j0_lo + (b_hi - b_lo)]
        wz = L - b_hi                      # trailing zeros (<= 386 <= ZW)
        if wz > 0:
            full[:, :, rows, b_hi:L] = zer[:, :, :, 0:wz]
        wl = b_lo                          # leading zeros (<= 446)
        if wl > 0:
            a = min(wl, ZW)
            full[:, :, rows, 0:a] = zer[:, :, :, 0:a]
            if wl > a:
                full[:, :, rows, a:wl] = zer[:, :, :, 0:wl - a]
    return full


def kernel(**inputs):
    return _run(inputs, trace=False)
